# revision 5
# baseline (speedup 1.0000x reference)
"""Causal self-attention (B=1, T=4096, C=1024, H=16, D=64) on 8 NeuronCores.

Sharding: tensor-parallel over heads. Core i handles heads (2i, 2i+1):
it computes q/k/v projections for its 128 qkv columns, attention for its
2 heads, and a partial output projection (rank-128 slice of the
contraction). The host sums the 8 partial outputs and adds b_proj.

Device layout notes:
  - x is transposed and cast to bf16 on the host, so the C (contraction)
    dim of every projection matmul is already on SBUF partitions.
  - scores are computed transposed (k on partitions, q free) so the
    attn @ v matmul needs no on-device transposes of the 4096^2 matrix.
  - softmax uses no max-subtraction (scores are ~N(0,1); exp is safe in
    f32/bf16) and the denominator comes from a ones-column appended to v.
"""

import sys

if "/opt/trn_rl_repo" not in sys.path:
    sys.path.insert(0, "/opt/trn_rl_repo")

import numpy as np
import ml_dtypes

T = 4096
C = 1024
H = 16
D = 64
NCORES = 8
HPC = H // NCORES  # heads per core = 2
QT = 512  # q-tile width
NQT = T // QT  # 8
KB = 128  # k-block
NKB = T // KB  # 32
BF16 = ml_dtypes.bfloat16

_COMPILED = None


def _build_nc():
    import concourse.tile as tile
    from concourse import bacc, mybir

    F32 = mybir.dt.float32
    BF = mybir.dt.bfloat16
    Exp = mybir.ActivationFunctionType.Exp

    nc = bacc.Bacc("TRN2", target_bir_lowering=False, debug=False,
                   num_devices=NCORES)

    def din(name, shape, dt=BF):
        return nc.dram_tensor(name, shape, dt, kind="ExternalInput").ap()

    xT = din("xT", [C, T])                 # x transposed, bf16
    wq = din("wq", [128, C])               # packed: [c%128, (c//128)*128 + m]
    wk = din("wk", [128, C])
    wv = din("wv", [128, C])
    wp = din("wp", [128, C])               # w_proj rows for this core's dims
    bq = din("bq", [1, 128])
    bk = din("bk", [1, 128])
    bv = din("bv", [1, 128])
    ones = din("ones", [1, QT])
    ident = din("ident", [128, 64])        # I64 stacked twice
    selA = din("selA", [1, 128])           # 1 for partitions 0-63
    selB = din("selB", [1, 128])           # 1 for partitions 64-127
    maskA = din("maskA", [128, 2 * QT])    # causal masks d=0 | d=128
    maskB = din("maskB", [128, 2 * QT])    # d=256 | d=384
    out = nc.dram_tensor("out", [T, C], F32, kind="ExternalOutput").ap()

    with tile.TileContext(nc) as tc:
        with (
            tc.tile_pool(name="const", bufs=1) as cpool,
            tc.tile_pool(name="qkv", bufs=1) as qkvpool,
            tc.tile_pool(name="exp", bufs=3) as epool,
            tc.tile_pool(name="small", bufs=2) as spool,
            tc.tile_pool(name="ostage", bufs=2) as opool,
            tc.tile_pool(name="ps_scores", bufs=1, space="PSUM") as ps_sc,
            tc.tile_pool(name="ps_avA", bufs=1, space="PSUM") as ps_avA,
            tc.tile_pool(name="ps_avB", bufs=1, space="PSUM") as ps_avB,
            tc.tile_pool(name="ps_shared", bufs=2, space="PSUM") as ps_sh,
        ):
            # ---- resident inputs ----
            xT_sb = cpool.tile([128, 8, T], BF, tag="xT")
            for c0 in range(8):
                nc.sync.dma_start(xT_sb[:, c0, :], xT[c0 * 128:(c0 + 1) * 128, :])
            w_sb = {}
            for nm, t in (("wq", wq), ("wk", wk), ("wv", wv), ("wp", wp)):
                w_sb[nm] = cpool.tile([128, C], BF, tag=nm, name=nm)
                nc.sync.dma_start(w_sb[nm][:], t[:])
            b_sb = {}
            for nm, t in (("bq", bq), ("bk", bk), ("bv", bv)):
                b_sb[nm] = cpool.tile([1, 128], BF, tag=nm, name=nm)
                nc.sync.dma_start(b_sb[nm][:], t[:])
            ones_sb = cpool.tile([1, QT], BF, tag="ones")
            nc.sync.dma_start(ones_sb[:], ones[:])
            ident_sb = cpool.tile([128, 64], BF, tag="ident")
            nc.sync.dma_start(ident_sb[:], ident[:])
            selA_sb = cpool.tile([1, 128], BF, tag="selA")
            nc.sync.dma_start(selA_sb[:], selA[:])
            selB_sb = cpool.tile([1, 128], BF, tag="selB")
            nc.sync.dma_start(selB_sb[:], selB[:])
            mA_sb = cpool.tile([128, 2 * QT], BF, tag="maskA")
            nc.sync.dma_start(mA_sb[:], maskA[:])
            mB_sb = cpool.tile([128, 2 * QT], BF, tag="maskB")
            nc.sync.dma_start(mB_sb[:], maskB[:])

            # ---- qkv projections: qT/kT/vT [128 (2 heads x 64), T] bf16 ----
            qT_sb = qkvpool.tile([128, T], BF, tag="qT")
            kT_sb = qkvpool.tile([128, T], BF, tag="kT")
            vT_sb = qkvpool.tile([128, T], BF, tag="vT")
            for wt, bias, dst in (("wq", "bq", qT_sb), ("wk", "bk", kT_sb),
                                  ("wv", "bv", vT_sb)):
                for j in range(NQT):
                    ps = ps_sh.tile([128, QT], F32, tag="sh")
                    for c0 in range(8):
                        nc.tensor.matmul(
                            ps[:],
                            lhsT=w_sb[wt][:, c0 * 128:(c0 + 1) * 128],
                            rhs=xT_sb[:, c0, j * QT:(j + 1) * QT],
                            start=(c0 == 0), stop=False)
                    nc.tensor.matmul(ps[:], lhsT=b_sb[bias][:],
                                     rhs=ones_sb[:], start=False, stop=True)
                    # qkv-phase copies go on ACT (idle before attention)
                    nc.scalar.copy(dst[:, j * QT:(j + 1) * QT], ps[:])

            # ---- v' blocks [128 k, 65] (col 64 = ones) per head ----
            vstore = []
            for h in range(2):
                vs = qkvpool.tile([128, NKB, 65], BF, tag=f"vst{h}", name=f"vst{h}")
                nc.gpsimd.memset(vs[:, :, 64], 1.0)
                for b in range(NKB):
                    pt = ps_sh.tile([128, 64], BF, tag="sh", name="pt")
                    nc.tensor.transpose(
                        pt[:, 0:64],
                        vT_sb[h * 64:(h + 1) * 64, b * 128:(b + 1) * 128],
                        ident_sb[h * 64:(h + 1) * 64, :])
                    nc.vector.tensor_copy(vs[:, b, 0:64], pt[:, 0:64])
                vstore.append(vs)

            # ---- attention + projection, per q-tile ----
            for i in range(NQT):
                avA = ps_avA.tile([128, QT], F32, tag="avA")
                avB = ps_avB.tile([128, QT], F32, tag="avB")
                npairs = 2 * (i + 1)
                nblk = 2 * npairs
                for p in range(npairs):
                    ps = ps_sc.tile([128, 4 * QT], F32, tag="sc")
                    for h in range(2):
                        for u in range(2):
                            b = 2 * p + u
                            nc.tensor.matmul(
                                ps[:, (2 * h + u) * QT:(2 * h + u + 1) * QT],
                                lhsT=kT_sb[h * 64:(h + 1) * 64,
                                           b * 128:(b + 1) * 128],
                                rhs=qT_sb[h * 64:(h + 1) * 64,
                                          i * QT:(i + 1) * QT],
                                start=True, stop=True)
                    et = epool.tile([128, 4 * QT], BF, tag="exp")
                    nc.scalar.activation(et[:], ps[:], Exp, scale=0.125)
                    if p == npairs - 2:
                        nc.vector.tensor_mul(et[:, 0:2 * QT],
                                             et[:, 0:2 * QT], mA_sb[:])
                        nc.vector.tensor_mul(et[:, 2 * QT:4 * QT],
                                             et[:, 2 * QT:4 * QT], mA_sb[:])
                    elif p == npairs - 1:
                        nc.vector.tensor_mul(et[:, 0:2 * QT],
                                             et[:, 0:2 * QT], mB_sb[:])
                        nc.vector.tensor_mul(et[:, 2 * QT:4 * QT],
                                             et[:, 2 * QT:4 * QT], mB_sb[:])
                    for h, av in ((0, avA), (1, avB)):
                        for u in range(2):
                            b = 2 * p + u
                            nc.tensor.matmul(
                                av[0:65, :],
                                lhsT=vstore[h][:, b, :],
                                rhs=et[:, (2 * h + u) * QT:(2 * h + u + 1) * QT],
                                start=(b == 0), stop=(b == nblk - 1))

                # softmax denominator -> reciprocal -> broadcast
                recipA = spool.tile([1, QT], F32, tag="recipA")
                recipB = spool.tile([1, QT], F32, tag="recipB")
                nc.vector.reciprocal(recipA[:], avA[64:65, :])
                nc.vector.reciprocal(recipB[:], avB[64:65, :])
                rbfA = spool.tile([1, QT], BF, tag="rbfA")
                rbfB = spool.tile([1, QT], BF, tag="rbfB")
                nc.vector.tensor_copy(rbfA[:], recipA[:])
                nc.vector.tensor_copy(rbfB[:], recipB[:])
                rbc_ps = ps_sh.tile([128, QT], F32, tag="sh", name="rbc_ps")
                nc.tensor.matmul(rbc_ps[:], lhsT=selA_sb[:], rhs=rbfA[:],
                                 start=True, stop=False)
                nc.tensor.matmul(rbc_ps[:], lhsT=selB_sb[:], rhs=rbfB[:],
                                 start=False, stop=True)
                rbc = spool.tile([128, QT], BF, tag="rbc")
                nc.vector.tensor_copy(rbc[:], rbc_ps[:])

                scaled = spool.tile([128, QT], BF, tag="scaled")
                nc.vector.tensor_mul(scaled[0:64, :], avA[0:64, :],
                                     rbc[0:64, :])
                nc.vector.tensor_mul(scaled[64:128, :], avB[0:64, :],
                                     rbc[64:128, :])

                # partial projection: out[q, :] += scaled^T.T @ wp
                for cchunk in range(4):
                    ost = opool.tile([128, C], F32, tag="ost")
                    lhs = scaled[:, cchunk * 128:(cchunk + 1) * 128]
                    for half in range(2):
                        pp = ps_sh.tile([128, QT], F32, tag="sh")
                        nc.tensor.matmul(
                            pp[:], lhsT=lhs,
                            rhs=w_sb["wp"][:, half * QT:(half + 1) * QT],
                            start=True, stop=True)
                        nc.vector.tensor_copy(
                            ost[:, half * QT:(half + 1) * QT], pp[:])
                    row = i * QT + cchunk * 128
                    nc.sync.dma_start(out[row:row + 128, :], ost[:])

    nc.compile()
    return nc


def _causal_mask(d):
    kp = np.arange(128)[:, None]
    qf = np.arange(QT)[None, :]
    return ((kp + d) <= qf).astype(BF16)


def _prep_inputs(x, w_qkv, b_qkv, w_proj):
    """Build the 8 per-core input maps (host-side shard + pack)."""
    xT = np.ascontiguousarray(x.reshape(T, C).T).astype(BF16)
    mA = np.concatenate([_causal_mask(0), _causal_mask(128)], axis=1)
    mB = np.concatenate([_causal_mask(256), _causal_mask(384)], axis=1)
    ident = np.zeros((128, 64), dtype=BF16)
    ident[np.arange(128), np.arange(128) % 64] = 1
    ones = np.ones((1, QT), dtype=BF16)
    selA = np.zeros((1, 128), dtype=BF16); selA[0, 0:64] = 1
    selB = np.zeros((1, 128), dtype=BF16); selB[0, 64:128] = 1

    def pack_w(wcols):  # [C, 128] -> [128, C] chunk-packed for SBUF
        return np.ascontiguousarray(
            wcols.reshape(8, 128, 128).transpose(1, 0, 2).reshape(128, C)
        ).astype(BF16)

    in_maps = []
    for core in range(NCORES):
        h0 = core * HPC
        cols = slice(h0 * D, (h0 + HPC) * D)  # 128 cols for this core
        m = {
            "xT": xT,
            "wq": pack_w(w_qkv[:, cols]),
            "wk": pack_w(w_qkv[:, C:][:, cols]),
            "wv": pack_w(w_qkv[:, 2 * C:][:, cols]),
            "wp": np.ascontiguousarray(w_proj[cols, :]).astype(BF16),
            "bq": b_qkv[cols].reshape(1, 128).astype(BF16),
            "bk": b_qkv[C:][cols].reshape(1, 128).astype(BF16),
            "bv": b_qkv[2 * C:][cols].reshape(1, 128).astype(BF16),
            "ones": ones,
            "ident": ident,
            "selA": selA,
            "selB": selB,
            "maskA": mA,
            "maskB": mB,
        }
        in_maps.append(m)
    return in_maps


def _get_compiled():
    global _COMPILED
    if _COMPILED is None:
        _COMPILED = _build_nc()
    return _COMPILED


def run_on_device(in_maps, **kwargs):
    from concourse.bass_utils import run_bass_kernel_spmd

    nc = _get_compiled()
    return run_bass_kernel_spmd(nc, in_maps, core_ids=list(range(NCORES)),
                                **kwargs)


def kernel(x, w_qkv, b_qkv, w_proj, b_proj, **run_kwargs):
    x = np.asarray(x, dtype=np.float32)
    w_qkv = np.asarray(w_qkv, dtype=np.float32)
    b_qkv = np.asarray(b_qkv, dtype=np.float32)
    w_proj = np.asarray(w_proj, dtype=np.float32)
    b_proj = np.asarray(b_proj, dtype=np.float32)

    in_maps = _prep_inputs(x, w_qkv, b_qkv, w_proj)
    res = run_on_device(in_maps, **run_kwargs)
    acc = np.zeros((T, C), dtype=np.float32)
    for core in range(NCORES):
        acc += res.results[core]["out"]
    acc += b_proj[None, :]
    out = acc.reshape(1, T, C)
    kernel.last_results = res
    return out


# revision 7
# speedup vs baseline: 1.2771x; 1.2771x over previous
"""Causal self-attention (B=1, T=4096, C=1024, H=16, D=64) on 8 NeuronCores.

Sharding: tensor-parallel over heads. Core i handles heads (2i, 2i+1):
it computes q/k/v projections for its 128 qkv columns, attention for its
2 heads, and a partial output projection (rank-128 slice of the
contraction). The host sums the 8 partial outputs and adds b_proj.

Device layout notes:
  - x is transposed and cast to bf16 on the host, so the C (contraction)
    dim of every projection matmul is already on SBUF partitions.
  - scores are computed transposed (k on partitions, q free) so the
    attn @ v matmul needs no on-device transposes of the 4096^2 matrix.
  - softmax uses no max-subtraction (scores are ~N(0,1); exp is safe in
    f32/bf16) and the denominator comes from a ones-column appended to v.
  - emission is software-pipelined: attn@v for block b is emitted after
    scores for block b+1, and the softmax/projection tail of q-tile i is
    emitted inside q-tile i+1's stream, so the PE FIFO never stalls on
    ACT/DVE results.
"""

import sys

if "/opt/trn_rl_repo" not in sys.path:
    sys.path.insert(0, "/opt/trn_rl_repo")

import numpy as np
import ml_dtypes

T = 4096
C = 1024
H = 16
D = 64
NCORES = 8
HPC = H // NCORES  # heads per core = 2
QT = 512  # q-tile width
NQT = T // QT  # 8
KB = 128  # k-block
NKB = T // KB  # 32
BF16 = ml_dtypes.bfloat16
OUT_BF16 = True  # partial outputs in bf16 (summed in f32 on host)

_COMPILED = None


def _build_nc():
    import concourse.tile as tile
    from concourse import bacc, mybir

    F32 = mybir.dt.float32
    BF = mybir.dt.bfloat16
    ODT = BF if OUT_BF16 else F32
    Exp = mybir.ActivationFunctionType.Exp

    nc = bacc.Bacc("TRN2", target_bir_lowering=False, debug=False,
                   num_devices=NCORES)

    def din(name, shape, dt=BF):
        return nc.dram_tensor(name, shape, dt, kind="ExternalInput").ap()

    xT = din("xT", [C, T])                 # x transposed, bf16
    wq = din("wq", [128, C])               # packed: [c%128, (c//128)*128 + m]
    wk = din("wk", [128, C])
    wv = din("wv", [128, C])
    wp = din("wp", [128, C])               # w_proj rows for this core's dims
    bq = din("bq", [1, 128])
    bk = din("bk", [1, 128])
    bv = din("bv", [1, 128])
    ones = din("ones", [1, QT])
    ident = din("ident", [128, 64])        # I64 stacked twice
    selA = din("selA", [1, 128])           # 1 for partitions 0-63
    selB = din("selB", [1, 128])           # 1 for partitions 64-127
    masks = [din(f"mask{d}", [128, 2 * QT]) for d in range(4)]  # [m_d | m_d]
    out = nc.dram_tensor("out", [T, C], ODT, kind="ExternalOutput").ap()

    with tile.TileContext(nc) as tc:
        with (
            tc.tile_pool(name="const", bufs=1) as cpool,
            tc.tile_pool(name="qkv", bufs=1) as qkvpool,
            tc.tile_pool(name="exp", bufs=4) as epool,
            tc.tile_pool(name="small", bufs=2) as spool,
            tc.tile_pool(name="ostage", bufs=2) as opool,
            tc.tile_pool(name="ps_main", bufs=3, space="PSUM") as ps_main,
            tc.tile_pool(name="ps_avA", bufs=1, space="PSUM") as ps_avA,
            tc.tile_pool(name="ps_avB", bufs=1, space="PSUM") as ps_avB,
        ):
            # ---- resident inputs ----
            xT_sb = cpool.tile([128, 8, T], BF, tag="xT")
            for c0 in range(8):
                nc.sync.dma_start(xT_sb[:, c0, :], xT[c0 * 128:(c0 + 1) * 128, :])
            w_sb = {}
            for nm, t in (("wq", wq), ("wk", wk), ("wv", wv), ("wp", wp)):
                w_sb[nm] = cpool.tile([128, C], BF, tag=nm, name=nm)
                nc.sync.dma_start(w_sb[nm][:], t[:])
            b_sb = {}
            for nm, t in (("bq", bq), ("bk", bk), ("bv", bv)):
                b_sb[nm] = cpool.tile([1, 128], BF, tag=nm, name=nm)
                nc.sync.dma_start(b_sb[nm][:], t[:])
            ones_sb = cpool.tile([1, QT], BF, tag="ones")
            nc.sync.dma_start(ones_sb[:], ones[:])
            ident_sb = cpool.tile([128, 64], BF, tag="ident")
            nc.sync.dma_start(ident_sb[:], ident[:])
            selA_sb = cpool.tile([1, 128], BF, tag="selA")
            nc.sync.dma_start(selA_sb[:], selA[:])
            selB_sb = cpool.tile([1, 128], BF, tag="selB")
            nc.sync.dma_start(selB_sb[:], selB[:])
            m_sb = []
            for d in range(4):
                mt = cpool.tile([128, 2 * QT], BF, tag=f"mask{d}",
                                name=f"mask{d}")
                nc.sync.dma_start(mt[:], masks[d][:])
                m_sb.append(mt)

            # ---- qkv projections: qT/kT/vT [128 (2 heads x 64), T] bf16 ----
            qT_sb = qkvpool.tile([128, T], BF, tag="qT")
            kT_sb = qkvpool.tile([128, T], BF, tag="kT")
            vT_sb = qkvpool.tile([128, T], BF, tag="vT")
            for wt, bias, dst in (("wv", "bv", vT_sb), ("wk", "bk", kT_sb),
                                  ("wq", "bq", qT_sb)):
                for j in range(NQT):
                    ps = ps_main.tile([128, QT], F32, tag="ps", name="psqkv")
                    for c0 in range(8):
                        nc.tensor.matmul(
                            ps[:],
                            lhsT=w_sb[wt][:, c0 * 128:(c0 + 1) * 128],
                            rhs=xT_sb[:, c0, j * QT:(j + 1) * QT],
                            start=(c0 == 0), stop=False)
                    nc.tensor.matmul(ps[:], lhsT=b_sb[bias][:],
                                     rhs=ones_sb[:], start=False, stop=True)
                    # qkv-phase copies go on ACT (idle before attention)
                    nc.scalar.copy(dst[:, j * QT:(j + 1) * QT], ps[:])

            # ---- v' blocks [128 k, 65] (col 64 = ones) per head ----
            vstore = []
            for h in range(2):
                vs = qkvpool.tile([128, NKB, 65], BF, tag=f"vst{h}",
                                  name=f"vst{h}")
                nc.gpsimd.memset(vs[:, :, 64], 1.0)
                for b in range(NKB):
                    pt = ps_main.tile([128, 64], BF, tag="ps", name="pt")
                    nc.tensor.transpose(
                        pt[:, 0:64],
                        vT_sb[h * 64:(h + 1) * 64, b * 128:(b + 1) * 128],
                        ident_sb[h * 64:(h + 1) * 64, :])
                    nc.vector.tensor_copy(vs[:, b, 0:64], pt[:, 0:64])
                vstore.append(vs)

            # ---- attention + projection, software-pipelined per q-tile ----
            def emit_scores(i, b):
                """scores block b for q-tile i -> exp -> mask; returns et."""
                ps = ps_main.tile([128, 2 * QT], F32, tag="ps", name="sc")
                for h in range(2):
                    nc.tensor.matmul(
                        ps[:, h * QT:(h + 1) * QT],
                        lhsT=kT_sb[h * 64:(h + 1) * 64,
                                   b * 128:(b + 1) * 128],
                        rhs=qT_sb[h * 64:(h + 1) * 64, i * QT:(i + 1) * QT],
                        start=True, stop=True)
                et = epool.tile([128, 2 * QT], BF, tag="exp", name="et")
                nc.scalar.activation(et[:], ps[:], Exp, scale=0.125)
                d = b - 4 * i  # diagonal-block offset /128
                if 0 <= d <= 3:
                    nc.vector.tensor_mul(et[:], et[:], m_sb[d][:])
                return et

            def emit_av(i, b, et, avA, avB, nblk):
                for h, av in ((0, avA), (1, avB)):
                    nc.tensor.matmul(
                        av[0:65, :],
                        lhsT=vstore[h][:, b, :],
                        rhs=et[:, h * QT:(h + 1) * QT],
                        start=(b == 0), stop=(b == nblk - 1))

            def tail_scale(i, avA, avB):
                recipA = spool.tile([1, QT], F32, tag="recipA", name="recipA")
                recipB = spool.tile([1, QT], F32, tag="recipB", name="recipB")
                nc.vector.reciprocal(recipA[:], avA[64:65, :])
                nc.vector.reciprocal(recipB[:], avB[64:65, :])
                rbfA = spool.tile([1, QT], BF, tag="rbfA", name="rbfA")
                rbfB = spool.tile([1, QT], BF, tag="rbfB", name="rbfB")
                nc.vector.tensor_copy(rbfA[:], recipA[:])
                nc.vector.tensor_copy(rbfB[:], recipB[:])
                rbc_ps = ps_main.tile([128, QT], F32, tag="ps", name="rbc_ps")
                nc.tensor.matmul(rbc_ps[:], lhsT=selA_sb[:], rhs=rbfA[:],
                                 start=True, stop=False)
                nc.tensor.matmul(rbc_ps[:], lhsT=selB_sb[:], rhs=rbfB[:],
                                 start=False, stop=True)
                rbc = spool.tile([128, QT], BF, tag="rbc", name="rbc")
                nc.vector.tensor_copy(rbc[:], rbc_ps[:])
                scaled = spool.tile([128, QT], BF, tag="scaled", name="scaled")
                nc.vector.tensor_mul(scaled[0:64, :], avA[0:64, :],
                                     rbc[0:64, :])
                nc.vector.tensor_mul(scaled[64:128, :], avB[0:64, :],
                                     rbc[64:128, :])
                return scaled

            def tail_proj(i, scaled):
                for cchunk in range(4):
                    pp = ps_main.tile([128, 2 * QT], F32, tag="ps", name="pp")
                    lhs = scaled[:, cchunk * 128:(cchunk + 1) * 128]
                    for half in range(2):
                        nc.tensor.matmul(
                            pp[:, half * QT:(half + 1) * QT], lhsT=lhs,
                            rhs=w_sb["wp"][:, half * QT:(half + 1) * QT],
                            start=True, stop=True)
                    ost = opool.tile([128, C], ODT, tag="ost", name="ost")
                    nc.vector.tensor_copy(ost[:], pp[:])
                    row = i * QT + cchunk * 128
                    nc.sync.dma_start(out[row:row + 128, :], ost[:])

            pend_scale = None  # (i, avA, avB) awaiting tail_scale/proj
            pend_proj = None
            for i in range(NQT):
                avA = ps_avA.tile([128, QT], F32, tag="avA", name="avA")
                avB = ps_avB.tile([128, QT], F32, tag="avB", name="avB")
                nblk = 4 * (i + 1)
                pend_av = None  # (b, et)
                for b in range(nblk):
                    et = emit_scores(i, b)
                    if b == 0 and pend_scale is not None:
                        pi, pA, pB = pend_scale
                        pscaled = tail_scale(pi, pA, pB)
                        pend_scale = None
                        pend_proj = (pi, pscaled)
                    if pend_av is not None:
                        emit_av(i, pend_av[0], pend_av[1], avA, avB, nblk)
                    if b == 1 and pend_proj is not None:
                        tail_proj(*pend_proj)
                        pend_proj = None
                    pend_av = (b, et)
                emit_av(i, pend_av[0], pend_av[1], avA, avB, nblk)
                pend_scale = (i, avA, avB)
                pend_proj = None
            # final tail
            pi, pA, pB = pend_scale
            scaled = tail_scale(pi, pA, pB)
            tail_proj(pi, scaled)

    nc.compile()
    return nc


def _causal_mask(d):
    kp = np.arange(128)[:, None]
    qf = np.arange(QT)[None, :]
    return ((kp + d) <= qf).astype(BF16)


def _prep_inputs(x, w_qkv, b_qkv, w_proj):
    """Build the 8 per-core input maps (host-side shard + pack)."""
    xT = np.ascontiguousarray(x.reshape(T, C).T).astype(BF16)
    masks = {}
    for d in range(4):
        m = _causal_mask(128 * d)
        masks[f"mask{d}"] = np.concatenate([m, m], axis=1)
    ident = np.zeros((128, 64), dtype=BF16)
    ident[np.arange(128), np.arange(128) % 64] = 1
    ones = np.ones((1, QT), dtype=BF16)
    selA = np.zeros((1, 128), dtype=BF16)
    selA[0, 0:64] = 1
    selB = np.zeros((1, 128), dtype=BF16)
    selB[0, 64:128] = 1

    def pack_w(wcols):  # [C, 128] -> [128, C] chunk-packed for SBUF
        return np.ascontiguousarray(
            wcols.reshape(8, 128, 128).transpose(1, 0, 2).reshape(128, C)
        ).astype(BF16)

    in_maps = []
    for core in range(NCORES):
        h0 = core * HPC
        cols = slice(h0 * D, (h0 + HPC) * D)  # 128 cols for this core
        m = {
            "xT": xT,
            "wq": pack_w(w_qkv[:, :C][:, cols]),
            "wk": pack_w(w_qkv[:, C:2 * C][:, cols]),
            "wv": pack_w(w_qkv[:, 2 * C:][:, cols]),
            "wp": np.ascontiguousarray(w_proj[cols, :]).astype(BF16),
            "bq": b_qkv[:C][cols].reshape(1, 128).astype(BF16),
            "bk": b_qkv[C:2 * C][cols].reshape(1, 128).astype(BF16),
            "bv": b_qkv[2 * C:][cols].reshape(1, 128).astype(BF16),
            "ones": ones,
            "ident": ident,
            "selA": selA,
            "selB": selB,
        }
        m.update(masks)
        in_maps.append(m)
    return in_maps


def _get_compiled():
    global _COMPILED
    if _COMPILED is None:
        _COMPILED = _build_nc()
    return _COMPILED


def run_on_device(in_maps, **kwargs):
    from concourse.bass_utils import run_bass_kernel_spmd

    nc = _get_compiled()
    return run_bass_kernel_spmd(nc, in_maps, core_ids=list(range(NCORES)),
                                **kwargs)


def kernel(x, w_qkv, b_qkv, w_proj, b_proj, **run_kwargs):
    x = np.asarray(x, dtype=np.float32)
    w_qkv = np.asarray(w_qkv, dtype=np.float32)
    b_qkv = np.asarray(b_qkv, dtype=np.float32)
    w_proj = np.asarray(w_proj, dtype=np.float32)
    b_proj = np.asarray(b_proj, dtype=np.float32)

    in_maps = _prep_inputs(x, w_qkv, b_qkv, w_proj)
    res = run_on_device(in_maps, **run_kwargs)
    acc = np.zeros((T, C), dtype=np.float32)
    for core in range(NCORES):
        acc += np.asarray(res.results[core]["out"], dtype=np.float32)
    acc += b_proj[None, :]
    out = acc.reshape(1, T, C)
    kernel.last_results = res
    return out
